# revision 21
# baseline (speedup 1.0000x reference)
"""Trainium2 Bass kernel for nn_MinLSTMCell (B=8, T=4096, D=1024, H=1024).

Self-contained: hardcodes shapes/sharding. Data-parallel over batch B across
8 NeuronCores (one batch element per core).

Math (verified against the reference):
  zf = x@Wf + bf, zi = x@Wi + bi, zh = x@Wh + bh
  u_h = exp(softplus(-zf) - softplus(-zi)) = (1 + e^{-zf}) * sigmoid(zi)
  g   = max(zh + 0.5, sigmoid(zh))         # = exp(log_g(zh))
  S_t = g0 + sum_{s<=t} u_h,s * g_s        # plain cumsum (a_star is not
                                           #  a running sum in the source)
  out[t] = S_t / (1 + u_h,t)               # f_t = 1/(1+u_h,t)
  out[0] = g0 = max(h0+0.5, sigmoid(h0))
Scaled form used on-chip (only exp/tanh/copy/identity act tables needed):
  q1 = 2*zh + 2bh + 1
  ef = e^{-zf-bf}; p = ef + 1; ti = tanh((zi+bi)/2)
  u  = (ti+1)*p = 2*u_h
  m1 = max(th+1, q1) = 2g   (th = tanh((q1-1)/4) = tanh((zh+bh)/2))
  w  = m1*u = 4*u_h*g;  S = 4*g0 + cumsum(w);  d = 2u+4
  host: out = S / d = S_true/(1+u_h)

Precision split (gate is 2e-2 on both absmax-normalized and pointwise):
  - zf, zi matmuls bf16 (u is pointwise-critical: fp8 would give ~0.09
    pointwise error at every t).
  - zh matmul: bf16 for t<512; fp8-e4m3 DoubleRow (2x PE rate) for t>=512.
    g-errors average out in the cumsum; only early-t points are pointwise-
    sensitive, hence the bf16 head.  Weights are pre-scaled by 64 (2*32) so
    uniform(-1/32,1/32) weights stay in e4m3 normal range; the 1/32 descale
    is folded into the ACT scale of q1.
  - Elementwise: ef/ti/p/u/d f32 (pointwise-critical), q1/th/m1/w bf16
    (scan-averaged), S f32.
"""


import numpy as np
import ml_dtypes

import concourse.mybir as mybir
import concourse.tile as tile
from concourse import bacc

B, T, D, H = 8, 4096, 1024, 1024
TB = 512            # t-block (psum free dim)
NTB = T // TB       # 8
NHT = H // 128      # 8 h-tiles of 128
NDK = D // 128      # 8 d-chunks (bf16)
NDC = D // 256      # 4 d-chunks (fp8 DoubleRow)
F32 = mybir.dt.float32
BF16 = mybir.dt.bfloat16
FP8 = mybir.dt.float8e4
AF = mybir.ActivationFunctionType
OP = mybir.AluOpType
DR = mybir.MatmulPerfMode.DoubleRow

NP_BF16 = ml_dtypes.bfloat16
NP_FP8 = ml_dtypes.float8_e4m3fn


def build_kernel():
    nc = bacc.Bacc()
    xb = nc.dram_tensor("xb", [128, NTB, NDK, TB], BF16, kind="ExternalInput")
    x8 = nc.dram_tensor("x8", [128, NTB, NDC, 2, TB], FP8, kind="ExternalInput")
    wf = nc.dram_tensor("wf", [128, NDK, H], BF16, kind="ExternalInput")
    wi = nc.dram_tensor("wi", [128, NDK, H], BF16, kind="ExternalInput")
    whb = nc.dram_tensor("whb", [128, NDK, H], BF16, kind="ExternalInput")  # 2*Wh
    wh8 = nc.dram_tensor("wh8", [128, NDC, 2, H], FP8, kind="ExternalInput")  # 64*Wh
    nbf = nc.dram_tensor("nbf", [128, NHT], F32, kind="ExternalInput")  # -bf
    hbi = nc.dram_tensor("hbi", [128, NHT], F32, kind="ExternalInput")  # bi/2
    b2h = nc.dram_tensor("b2h", [128, NHT], F32, kind="ExternalInput")  # 2bh+1
    g4v = nc.dram_tensor("g4v", [128, NHT], F32, kind="ExternalInput")  # 4*g0
    s_out = nc.dram_tensor("s_out", [H, T], BF16, kind="ExternalOutput")
    d_out = nc.dram_tensor("d_out", [H, T], BF16, kind="ExternalOutput")

    with tile.TileContext(nc) as tc:
        with (
            tc.tile_pool(name="singles", bufs=1) as singles,
            tc.tile_pool(name="xbp", bufs=4) as xb_p,
            tc.tile_pool(name="x8p", bufs=4) as x8_p,
            tc.tile_pool(name="pz", bufs=8, space="PSUM") as pz,
            tc.tile_pool(name="ew", bufs=3) as ew,
            tc.tile_pool(name="scan", bufs=9) as scan_p,
        ):
            engs = [nc.sync, nc.scalar, nc.gpsimd]

            def vload(name, dram):
                t = singles.tile([128, NHT], F32, tag=name)
                nc.scalar.dma_start(t[:], dram[:])
                return t

            nbf_t = vload("nbf", nbf)
            hbi_t = vload("hbi", hbi)
            b2h_t = vload("b2h", b2h)
            g4v_t = vload("g4v", g4v)
            c25_t = singles.tile([128, 1], F32, tag="c25")
            nc.vector.memset(c25_t[:], -0.25)

            # scratch for PE warmup (HAM un-throttle): garbage-in dummy
            # matmuls keep the PE busy while the first weights stream in.
            wdum = singles.tile([128, 128], BF16, tag="wdum")
            nc.vector.memset(wdum[:], 0.0)
            sdum = singles.tile([128, TB], BF16, tag="sdum")
            nc.vector.memset(sdum[:], 0.0)
            pdum = pz.tile([128, TB], F32, tag="z")
            for _ in range(24):
                nc.tensor.matmul(pdum[:], wdum[:], sdum[:], start=True, stop=True)

            # ---- startup DMA: whole-tensor transfers (>=1MB hits full HBM
            # rate; small tiles are descriptor-dominated).  sync carries the
            # three 2MB bf16 weight blobs in tb0 gate order; scalar carries
            # xb(tb0) + wh8; gpsimd carries the tb1 x blocks.
            def xload_b(tb):
                t = xb_p.tile([128, NDK, TB], BF16, tag="xB")
                for k in range(NDK):
                    eng = nc.scalar if k < 6 else nc.sync
                    eng.dma_start(t[:, k, :], xb[:, tb, k, :])
                return t

            def xload_8(tb):
                t = x8_p.tile([128, NDC, 2, TB], FP8, tag="x8")
                for c in range(NDC):
                    eng = nc.scalar if c < 2 else nc.gpsimd
                    eng.dma_start(t[:, c, :, :], x8[:, tb, c, :, :])
                return t

            n = 0

            def dma(dst, srcap):
                nonlocal n
                engs[n % 3].dma_start(dst, srcap)
                n += 1

            wf_sb = singles.tile([128, NDK, H], BF16, tag="wf")
            wi_sb = singles.tile([128, NDK, H], BF16, tag="wi")
            whb_sb = singles.tile([128, NDK, H], BF16, tag="whb")
            x_cur = xb_p.tile([128, NDK, TB], BF16, tag="xB")
            for k in range(NDK):
                dma(x_cur[:, k, :], xb[:, 0, k, :])
                dma(wf_sb[:, k, :], wf[:, k, :])
            for k in range(NDK):
                dma(wi_sb[:, k, :], wi[:, k, :])
            for k in range(NDK):
                dma(whb_sb[:, k, :], whb[:, k, :])
            wh8_sb = singles.tile([128, NDC, 2, H], FP8, tag="wh8")
            for c in range(NDC):
                dma(wh8_sb[:, c, :, :], wh8[:, c, :, :])
            x8_nxt = x8_p.tile([128, NDC, 2, TB], FP8, tag="x8")
            for c in range(NDC):
                dma(x8_nxt[:, c, :, :], x8[:, 1, c, :, :])
            xb_nxt = xb_p.tile([128, NDK, TB], BF16, tag="xB")
            for k in range(NDK):
                dma(xb_nxt[:, k, :], xb[:, 1, k, :])
            def emit_xload_pair(tb):
                return xload_b(tb), xload_8(tb)

            xq = [(xb_nxt, x8_nxt)]

            s_prev = [None] * NHT
            pending = None  # (w, s_tag carry info) -> scan pipelined 1 behind
            x8_cur = None

            def emit_tail(ht, tb, t0, hs, zh, zf, zi, ef, ti, q1scale):
                """zh just landed for (tb, ht): emit the elementwise chain."""
                nonlocal pending
                q1 = ew.tile([128, TB], BF16, tag="q1")
                nc.scalar.activation(
                    q1[:], zh[:], AF.Identity,
                    bias=b2h_t[:, ht:ht + 1], scale=q1scale)
                th = ew.tile([128, TB], BF16, tag="th")
                nc.scalar.activation(
                    th[:], q1[:], AF.Tanh, bias=c25_t[:, 0:1], scale=0.25)
                if ef is None:
                    ef = ew.tile([128, TB], F32, tag="ef", bufs=8)
                    nc.scalar.activation(
                        ef[:], zf[:], AF.Exp,
                        bias=nbf_t[:, ht:ht + 1], scale=-1.0)
                if ti is None:
                    ti = ew.tile([128, TB], F32, tag="ti", bufs=8)
                    nc.scalar.activation(
                        ti[:], zi[:], AF.Tanh,
                        bias=hbi_t[:, ht:ht + 1], scale=0.5)
                p = ew.tile([128, TB], F32, tag="p")
                nc.scalar.activation(p[:], ef[:], AF.Copy, bias=1.0)
                # ---- DVE
                m1 = ew.tile([128, TB], BF16, tag="m1")
                nc.vector.scalar_tensor_tensor(
                    m1[:], th[:], 1.0, q1[:], op0=OP.add, op1=OP.max)
                u = ew.tile([128, TB], F32, tag="u")
                nc.vector.scalar_tensor_tensor(
                    u[:], ti[:], 1.0, p[:], op0=OP.add, op1=OP.mult)
                d = ew.tile([128, TB], BF16, tag="d")
                nc.vector.tensor_scalar(
                    d[:], u[:], 2.0, 4.0, op0=OP.mult, op1=OP.add)
                deng = engs[(2 * ht) % 3] if tb == NTB - 1 else nc.gpsimd
                deng.dma_start(d_out[hs, t0:t0 + TB], d[:])
                # ---- GPSIMD
                w = ew.tile([128, TB], BF16, tag="w")
                nc.gpsimd.tensor_mul(w[:], m1[:], u[:])
                # ---- scan pipelined one tile behind (breaks the
                # DVE-waits-on-GPSIMD in-order stall)
                if pending is not None:
                    pw, pht, ptb, pt0, phs = pending
                    s_t = scan_p.tile([128, TB], F32, tag="S")
                    init = (
                        g4v_t[:, pht:pht + 1] if ptb == 0
                        else s_prev[pht][:, TB - 1:TB])
                    nc.vector.tensor_tensor_scan(
                        s_t[:], pw[:], pw[:], initial=init,
                        op0=OP.add, op1=OP.bypass)
                    s_prev[pht] = s_t
                    s8 = ew.tile([128, TB], BF16, tag="s8")
                    nc.vector.tensor_copy(s8[:], s_t[:])
                    seng = engs[(2 * pht + 1) % 3] if ptb == NTB - 1 else nc.sync
                    seng.dma_start(s_out[phs, pt0:pt0 + TB], s8[:])
                pending = (w, ht, tb, t0, hs)

            # ---- tb0: bf16 head block, gate-major so the PE ramps with the
            # DMA arrival order (wf -> wi -> whb) instead of stalling the
            # in-order PE queue on ht0's zi/zh.
            xB = x_cur
            ef_t, ti_t = [], []
            zf_t = [pz.tile([128, TB], F32, tag="z", name=f"zf{h}")
                    for h in range(NHT)]
            for k in range(NDK):
                for ht in range(NHT):
                    hs = slice(ht * 128, (ht + 1) * 128)
                    nc.tensor.matmul(
                        zf_t[ht][:], wf_sb[:, k, hs], xB[:, k, :],
                        start=(k == 0), stop=(k == NDK - 1))
            for ht in range(NHT):
                ef = ew.tile([128, TB], F32, tag="ef", bufs=8)
                nc.scalar.activation(
                    ef[:], zf_t[ht][:], AF.Exp,
                    bias=nbf_t[:, ht:ht + 1], scale=-1.0)
                ef_t.append(ef)
            zi_t = [pz.tile([128, TB], F32, tag="z", name=f"zi{h}")
                    for h in range(NHT)]
            for k in range(NDK):
                for ht in range(NHT):
                    hs = slice(ht * 128, (ht + 1) * 128)
                    nc.tensor.matmul(
                        zi_t[ht][:], wi_sb[:, k, hs], xB[:, k, :],
                        start=(k == 0), stop=(k == NDK - 1))
            for ht in range(NHT):
                ti = ew.tile([128, TB], F32, tag="ti", bufs=8)
                nc.scalar.activation(
                    ti[:], zi_t[ht][:], AF.Tanh,
                    bias=hbi_t[:, ht:ht + 1], scale=0.5)
                ti_t.append(ti)
            for ht in range(NHT):
                hs = slice(ht * 128, (ht + 1) * 128)
                zh = pz.tile([128, TB], F32, tag="z")
                for k in range(NDK):
                    nc.tensor.matmul(
                        zh[:], whb_sb[:, k, hs], xB[:, k, :],
                        start=(k == 0), stop=(k == NDK - 1))
                if ht == 0:
                    x_cur, x8_cur = xq.pop(0)
                    xq.append(emit_xload_pair(2))
                emit_tail(ht, 0, 0, hs, zh, None, None,
                          ef_t[ht], ti_t[ht], 1.0)

            # ---- tb1..7: fp8 DoubleRow zh, ht-major
            for tb in range(1, NTB):
                t0 = tb * TB
                xB = x_cur
                x8B = x8_cur
                for ht in range(NHT):
                    hs = slice(ht * 128, (ht + 1) * 128)
                    # fp8 DoubleRow zh first (longest elementwise chain)
                    zh = pz.tile([128, TB], F32, tag="z")
                    for c in range(NDC):
                        nc.tensor.matmul(
                            zh[:], wh8_sb[:, c, :, hs], x8B[:, c, :, :],
                            start=(c == 0), stop=(c == NDC - 1),
                            perf_mode=DR)
                    zf = pz.tile([128, TB], F32, tag="z")
                    for k in range(NDK):
                        nc.tensor.matmul(
                            zf[:], wf_sb[:, k, hs], xB[:, k, :],
                            start=(k == 0), stop=(k == NDK - 1))
                    zi = pz.tile([128, TB], F32, tag="z")
                    for k in range(NDK):
                        nc.tensor.matmul(
                            zi[:], wi_sb[:, k, hs], xB[:, k, :],
                            start=(k == 0), stop=(k == NDK - 1))
                    # prefetch next block's x during ht 0
                    if ht == 0 and tb + 1 < NTB:
                        x_cur, x8_cur = xq.pop(0)
                        if tb + 2 < NTB:
                            xq.append(emit_xload_pair(tb + 2))
                    emit_tail(ht, tb, t0, hs, zh, zf, zi,
                              None, None, 1.0 / 32.0)
            # drain the last tile
            pw, pht, ptb, pt0, phs = pending
            s_t = scan_p.tile([128, TB], F32, tag="S")
            init = s_prev[pht][:, TB - 1:TB]
            nc.vector.tensor_tensor_scan(
                s_t[:], pw[:], pw[:], initial=init, op0=OP.add, op1=OP.bypass)
            s8 = ew.tile([128, TB], BF16, tag="s8")
            nc.vector.tensor_copy(s8[:], s_t[:])
            nc.sync.dma_start(s_out[phs, pt0:pt0 + TB], s8[:])
    nc.finalize()
    return nc


_NC_CACHE = None


def get_nc():
    global _NC_CACHE
    if _NC_CACHE is None:
        _NC_CACHE = build_kernel()
    return _NC_CACHE


def prep_in_maps(x_t, h_prev, Wf, bf, Wi, bi, Wh, bh):
    x_t = np.asarray(x_t, dtype=np.float32)
    h_prev = np.asarray(h_prev, dtype=np.float32)
    Wf = np.asarray(Wf, dtype=np.float32)
    Wi = np.asarray(Wi, dtype=np.float32)
    Wh = np.asarray(Wh, dtype=np.float32)
    bf = np.asarray(bf, dtype=np.float32)
    bi = np.asarray(bi, dtype=np.float32)
    bh = np.asarray(bh, dtype=np.float32)

    g0 = np.maximum(h_prev + 0.5, 1.0 / (1.0 + np.exp(-h_prev))).astype(np.float32)

    def wpack(W):
        # (p, k, h) = W[k*128+p, h]
        return np.ascontiguousarray(
            W.reshape(NDK, 128, H).transpose(1, 0, 2).astype(NP_BF16))

    wf_b = wpack(Wf)
    wi_b = wpack(Wi)
    whb_ = wpack(2.0 * Wh)
    # fp8 weights: (p, c, j, m) = 64*Wh[c*256 + j*128 + p, m]
    wh8_ = np.ascontiguousarray(
        (64.0 * Wh).reshape(NDC, 2, 128, H).transpose(2, 0, 1, 3)
        .astype(NP_FP8))

    nbf = np.ascontiguousarray((-bf).reshape(NHT, 128).T)
    hbi = np.ascontiguousarray((0.5 * bi).reshape(NHT, 128).T)
    b2h = np.ascontiguousarray((2.0 * bh + 1.0).reshape(NHT, 128).T)

    in_maps = []
    for b in range(B):
        xT = np.ascontiguousarray(x_t[b].T)                       # [D, T] f32
        # (p, tb, k, t) = x[k*128+p, tb*TB+t]
        xb_ = np.ascontiguousarray(
            xT.reshape(NDK, 128, NTB, TB).transpose(1, 2, 0, 3).astype(NP_BF16))
        # (p, tb, c, j, t) = x[c*256+j*128+p, tb*TB+t]
        x8_ = np.ascontiguousarray(
            xT.reshape(NDC, 2, 128, NTB, TB)
            .transpose(2, 3, 0, 1, 4).astype(NP_FP8))
        g4v = np.ascontiguousarray((4.0 * g0[b]).reshape(NHT, 128).T)
        in_maps.append({
            "xb": xb_, "x8": x8_,
            "wf": wf_b, "wi": wi_b, "whb": whb_, "wh8": wh8_,
            "nbf": nbf, "hbi": hbi, "b2h": b2h,
            "g4v": g4v,
        })
    return in_maps, g0


def kernel(x_t, h_prev, Wf, bf, Wi, bi, Wh, bh, _run_opts=None):
    from concourse.bass_utils import run_bass_kernel_spmd

    in_maps, g0 = prep_in_maps(x_t, h_prev, Wf, bf, Wi, bi, Wh, bh)
    nc = get_nc()

    opts = _run_opts or {}
    res = run_bass_kernel_spmd(nc, in_maps, core_ids=list(range(B)), **opts)

    out = np.empty((B, T + 1, H), dtype=np.float32)
    for b in range(B):
        out[b, 0, :] = g0[b]
        S = res.results[b]["s_out"].astype(np.float32)
        dd = res.results[b]["d_out"].astype(np.float32)
        out[b, 1:, :] = (S / dd).T
    if _run_opts is not None:
        return out, res
    return out
